# revision 11
# baseline (speedup 1.0000x reference)
"""Trainium2 Bass kernel v3 for nn_BaselineGRU: 2-layer GRU (B=16,T=64,NN=4096,
H=1024) + decoder on 8 NeuronCores.

v3 over v2:
- interleaved exchange layout (recvf cols r*32+u*16+b): ONE send DMA and ONE
  recv DMA per slot instead of four.
- gx (r,z gates) + both biases folded into the per-step PSUM via identity
  matmuls: kills the DVE pre-add on the critical path; sigmoid reads PSUM.
- gates produce h' directly in bf16 into the send tile (no separate cast);
  h_old is read back from the previous send tile (bf16).
- gx0g / gxs staged in bf16 (halves SBUF + enables the identity-MM fold).
"""
import numpy as np

import concourse.bacc as bacc
import concourse.tile as tile
import concourse.mybir as mybir
from concourse import bass_utils

B, T, NN, H = 16, 64, 4096, 1024
NCR = 8
HC = H // NCR        # 128 hidden per core
KH = H // 128        # 8 K-chunks over hidden
KX = NN // 128       # 32 K-chunks over input features
DC = NN // NCR       # 512 decoder rows per core
TB = T * B           # 1024 tokens
fp32 = mybir.dt.float32
bf16 = mybir.dt.bfloat16

_CACHE = {}


def _build(TT=T):
    nc = bacc.Bacc("TRN2", target_bir_lowering=False, debug=False,
                   enable_asserts=False, num_devices=NCR)
    S = mybir.ActivationFunctionType.Sigmoid
    TA = mybir.ActivationFunctionType.Tanh
    d = {}
    d["xd"] = nc.dram_tensor("xd", [KX, 128, TB], bf16, kind="ExternalInput").ap()
    d["wih0T"] = nc.dram_tensor("wih0T", [128, 3 * KX * 128], bf16,
                                kind="ExternalInput").ap()
    d["whh0T"] = nc.dram_tensor("whh0T", [128, 3 * KH * 128], bf16,
                                kind="ExternalInput").ap()
    d["wih1T"] = nc.dram_tensor("wih1T", [128, 3 * KH * 128], bf16,
                                kind="ExternalInput").ap()
    d["whh1T"] = nc.dram_tensor("whh1T", [128, 3 * KH * 128], bf16,
                                kind="ExternalInput").ap()
    d["decw"] = nc.dram_tensor("decw", [128, KH * DC], bf16,
                               kind="ExternalInput").ap()
    # bih0 / b1x: per-gate packed biases for the input projections
    #   (r,z: b_ih+b_hh; n: b_ih only).  b0hn / b1hn: b_hh n-gate slice.
    for nm in ("bih0", "b1x"):
        d[nm] = nc.dram_tensor(nm, [1, 3 * 128], bf16, kind="ExternalInput").ap()
    for nm in ("b0hn", "b1hn"):
        d[nm] = nc.dram_tensor(nm, [1, 128], bf16, kind="ExternalInput").ap()
    d["decb"] = nc.dram_tensor("decb", [1, DC], bf16, kind="ExternalInput").ap()
    d["ones"] = nc.dram_tensor("ones", [1, 512], bf16, kind="ExternalInput").ap()
    d["ident"] = nc.dram_tensor("ident", [128, 128], bf16,
                                kind="ExternalInput").ap()
    out_d = nc.dram_tensor("out", [B, DC], fp32, kind="ExternalOutput").ap()

    with tile.TileContext(nc) as tc:
        with tc.tile_pool(name="wsb", bufs=1) as wsb, \
             tc.tile_pool(name="gp", bufs=3) as gp, \
             tc.tile_pool(name="agd", bufs=4, space="DRAM") as agd:

            # ---- persistent SBUF ----
            wih0T = wsb.tile([128, 3 * KX * 128], bf16, tag="wih0T")
            whh0T = wsb.tile([128, 3 * KH * 128], bf16, tag="whh0T")
            wih1T = wsb.tile([128, 3 * KH * 128], bf16, tag="wih1T")
            whh1T = wsb.tile([128, 3 * KH * 128], bf16, tag="whh1T")
            decw = wsb.tile([128, KH * DC], bf16, tag="decw")
            ident = wsb.tile([128, 128], bf16, tag="ident")
            for t_, nm in ((wih0T, "wih0T"), (whh0T, "whh0T"), (wih1T, "wih1T"),
                           (whh1T, "whh1T"), (decw, "decw"), (ident, "ident")):
                nc.sync.dma_start(out=t_[:], in_=d[nm])
            bias = {}
            for nm in ("bih0", "b1x"):
                bias[nm] = wsb.tile([1, 3 * 128], bf16, tag=nm, name=nm)
                nc.sync.dma_start(out=bias[nm][:], in_=d[nm])
            for nm in ("b0hn", "b1hn"):
                bias[nm] = wsb.tile([1, 128], bf16, tag=nm, name=nm)
                nc.sync.dma_start(out=bias[nm][:], in_=d[nm])
            decb = wsb.tile([1, DC], bf16, tag="decb")
            nc.sync.dma_start(out=decb[:], in_=d["decb"])
            ones = wsb.tile([1, 512], bf16, tag="ones")
            nc.sync.dma_start(out=ones[:], in_=d["ones"])

            gx0g = [wsb.tile([128, 16 * T], bf16, tag=f"gx0g{g}",
                             name=f"gx0g{g}") for g in range(3)]
            gxs = [wsb.tile([128, 48], bf16, tag=f"gxs{p}", name=f"gxs{p}")
                   for p in range(2)]
            # recv split into two TILES so Tile tracks h1/h2 deps separately:
            # recv1 cols r*16+b hold h1 chunks, recv2 same for h2.
            recv1 = [wsb.tile([128, 128], bf16, tag=f"recv1_{p}",
                              name=f"recv1_{p}") for p in range(3)]
            recv2 = [wsb.tile([128, 128], bf16, tag=f"recv2_{p}",
                              name=f"recv2_{p}") for p in range(3)]
            # sendf cols: u*16 + b
            sendf = [wsb.tile([128, 32], bf16, tag=f"sendf{p}",
                              name=f"sendf{p}") for p in range(2)]
            for t_ in recv1 + recv2 + sendf:
                nc.vector.memset(t_[:], 0.0)

            agoutS = [nc.dram_tensor(f"agoutS{j}", [128 * NCR, 32], bf16,
                                     kind="Internal", addr_space="Shared")
                      for j in range(4)]

            def emit_fused_ag(s):
                """One AG per slot; recv unpacked into the two tiles, h1
                first so ps0 can start before the h2 half lands."""
                agin = agd.tile([128, 32], bf16, tag="agin", name="agin")
                nc.sync.dma_start(out=agin[:], in_=sendf[s % 2][:])
                ago = agoutS[s % 4].ap()
                nc.gpsimd.collective_compute(
                    "AllGather", mybir.AluOpType.bypass,
                    replica_groups=[list(range(NCR))],
                    ins=[agin.opt()], outs=[ago])
                a3 = ago.rearrange("(r p) c -> p r c", p=128)
                nc.sync.dma_start(
                    out=recv1[s % 3][:].rearrange("p (r b) -> p r b", r=NCR),
                    in_=a3[:, :, 0:16])
                nc.sync.dma_start(
                    out=recv2[s % 3][:].rearrange("p (r b) -> p r b", r=NCR),
                    in_=a3[:, :, 16:32])

            # ---- layer-0 input projection: gx0g[g][:, 16s+b] ----
            with tc.tile_pool(name="ppj", bufs=1, space="PSUM") as ppj, \
                 tc.tile_pool(name="xp", bufs=2) as xp:
                pj = []
                for g in range(3):
                    for hf in range(2):
                        t_ = ppj.tile([128, 512], fp32, tag=f"pj{g}{hf}",
                                      name=f"pj{g}{hf}")
                        pj.append(t_)
                        nc.tensor.matmul(t_[:],
                                         bias["bih0"][:, 128 * g:128 * (g + 1)],
                                         ones[0:1, 0:512], start=True, stop=False)
                for k in range(KX):
                    xt = xp.tile([128, TB], bf16, tag="xt")
                    nc.sync.dma_start(out=xt[:], in_=d["xd"][k])
                    for g in range(3):
                        for hf in range(2):
                            nc.tensor.matmul(
                                pj[2 * g + hf][:],
                                wih0T[:, (g * KX + k) * 128:(g * KX + k + 1) * 128],
                                xt[:, 512 * hf:512 * (hf + 1)],
                                start=False, stop=(k == KX - 1))
                for g in range(3):
                    for hf in range(2):
                        nc.vector.tensor_copy(gx0g[g][:, 512 * hf:512 * (hf + 1)],
                                              pj[2 * g + hf][:])

            from contextlib import ExitStack
            _pss = ExitStack()
            psp = _pss.enter_context(
                tc.tile_pool(name="psp", bufs=1, space="PSUM"))

            # ---- gates (transposed layout). ps: rz cols already hold
            # gh+gx+biases; n col holds gh+b_hn. ----
            def gates_T(tag, ps, gxn, h_old, h_out):
                rz = gp.tile([128, 32], fp32, tag=f"{tag}rz")
                nc.scalar.activation(rz[:], ps[:, 0:32], S)
                c0 = gp.tile([128, 16], fp32, tag=f"{tag}c0")
                nc.vector.tensor_mul(c0[:], rz[:, 0:16], ps[:, 32:48])
                d0 = gp.tile([128, 16], fp32, tag=f"{tag}d0")
                nc.vector.tensor_add(d0[:], gxn, c0[:])
                n0 = gp.tile([128, 16], fp32, tag=f"{tag}n0")
                nc.scalar.activation(n0[:], d0[:], TA)
                e0 = gp.tile([128, 16], fp32, tag=f"{tag}e0")
                nc.vector.tensor_sub(e0[:], h_old, n0[:])
                f0 = gp.tile([128, 16], fp32, tag=f"{tag}f0")
                nc.vector.tensor_mul(f0[:], rz[:, 16:32], e0[:])
                nc.vector.tensor_add(h_out, n0[:], f0[:])

            def gemm48(ps, wT, gx_r, gx_z, bias_n, rvt, with_h):
                """rz: ps[:,0:32] = gx (ident MM, carries biases) + gh.
                   n:  ps[:,32:48] = bias_n + gh_n.
                   Each gate's accumulation group is emitted contiguously —
                   interleaving other regions' start-MMs breaks the group."""
                leads = [(0, ident[:], gx_r), (1, ident[:], gx_z),
                         (2, bias_n, ones[0:1, 0:16])]
                for g, lh, rh in leads:
                    nc.tensor.matmul(ps[:, 16 * g:16 * (g + 1)], lh, rh,
                                     start=True, stop=not with_h)
                    if with_h:
                        for k in range(KH):
                            nc.tensor.matmul(
                                ps[:, 16 * g:16 * (g + 1)],
                                wT[:, (g * KH + k) * 128:(g * KH + k + 1) * 128],
                                rvt[:, 16 * k:16 * (k + 1)],
                                start=False, stop=(k == KH - 1))

            # ---- scan: slots 0..T+2 ----
            T_ = TT
            for s in range(T_ + 3):
                rv1 = recv1[(s - 1) % 3]
                rv2 = recv2[(s - 1) % 3]
                # chain-1 recurrent GEMM first (critical path)
                if s < T_:
                    ps0 = psp.tile([128, 48], fp32, tag=f"ps0{s % 2}",
                                   name=f"ps0{s % 2}")
                    gemm48(ps0, whh0T,
                           gx0g[0][:, 16 * s:16 * (s + 1)],
                           gx0g[1][:, 16 * s:16 * (s + 1)],
                           bias["b0hn"], rv1, s >= 1)
                # chain-2 recurrent GEMM: produces h2(s-2)
                if 2 <= s <= T_ + 1:
                    psh = psp.tile([128, 48], fp32, tag=f"psh{s % 2}",
                                   name=f"psh{s % 2}")
                    gemm48(psh, whh1T, gxs[s % 2][:, 0:16], gxs[s % 2][:, 16:32],
                           bias["b1hn"], rv2, s >= 3)
                # chain-2 input GEMM prefetch: gx1 for h2(s-1), used next slot
                if 1 <= s <= T_:
                    psx = psp.tile([128, 48], fp32, tag=f"psx{s % 2}",
                                   name=f"psx{s % 2}")
                    # gx1 = wih1 @ h1(s-1) + biases (r,z: both; n: b_ih only)
                    for g in range(3):
                        nc.tensor.matmul(psx[:, 16 * g:16 * (g + 1)],
                                         bias["b1x"][:, 128 * g:128 * (g + 1)],
                                         ones[0:1, 0:16], start=True, stop=False)
                        for k in range(KH):
                            nc.tensor.matmul(
                                psx[:, 16 * g:16 * (g + 1)],
                                wih1T[:, (g * KH + k) * 128:(g * KH + k + 1) * 128],
                                rv1[:, 16 * k:16 * (k + 1)],
                                start=False, stop=(k == KH - 1))

                # chain-1 gates -> h1(s) into sendf bf16
                if s < T_:
                    gates_T("l0", ps0, gx0g[2][:, 16 * s:16 * (s + 1)],
                            sendf[(s - 1) % 2][:, 0:16],
                            sendf[s % 2][:, 0:16])
                # chain-2 gates -> h2(s-2) into sendf bf16
                if 2 <= s <= T_ + 1:
                    gates_T("l1", psh, gxs[s % 2][:, 32:48],
                            sendf[(s - 1) % 2][:, 16:32],
                            sendf[s % 2][:, 16:32])
                if s <= T_ + 1:
                    emit_fused_ag(s)
                # evict psx to SBUF (bf16) for next slot's chain-2
                if 1 <= s <= T_:
                    nc.scalar.activation(gxs[(s + 1) % 2][:], psx[:],
                                         mybir.ActivationFunctionType.Copy)

            # ---- decoder: out = h2(T-1) @ dec_w_c.T + dec_b_c ----
            rvt = recv2[(T_ + 1) % 3]
            pd = psp.tile([16, DC], fp32, tag="dec")
            for k in range(KH):
                nc.tensor.matmul(pd[:], rvt[:, 16 * k:16 * (k + 1)],
                                 decw[:, k * DC:(k + 1) * DC],
                                 start=(k == 0), stop=False)
            nc.tensor.matmul(pd[:], ones[0:1, 0:16], decb[:],
                             start=False, stop=True)
            od = gp.tile([16, DC], fp32, tag="od")
            nc.vector.tensor_copy(od[:], pd[:])
            nc.sync.dma_start(out=out_d, in_=od[:])
            _pss.close()

    nc.compile()
    return nc


def _prep_in_maps(x, w_ih_l0, w_hh_l0, b_ih_l0, b_hh_l0,
                  w_ih_l1, w_hh_l1, b_ih_l1, b_hh_l1, dec_w, dec_b):
    bfnp = mybir.dt.np(bf16)
    x = np.asarray(x, np.float32)
    xt = np.ascontiguousarray(x.transpose(2, 1, 0).reshape(NN, TB))
    xd = np.ascontiguousarray(xt.reshape(KX, 128, TB)).astype(bfnp)

    def pack_T(w, c, kchunks):
        w = np.asarray(w, np.float32)
        out = np.empty((128, 3 * kchunks * 128), np.float32)
        for g in range(3):
            wg = w[g * H + c * HC: g * H + (c + 1) * HC, :]
            blk = wg.reshape(128, kchunks, 128).transpose(2, 1, 0)
            out[:, g * kchunks * 128:(g + 1) * kchunks * 128] = \
                blk.reshape(128, kchunks * 128)
        return out.astype(bfnp)

    def bias_mix(bi, bh, c):
        """[r: bi+bh, z: bi+bh, n: bi] packed [1, 384]."""
        bi = np.asarray(bi, np.float32)
        bh = np.asarray(bh, np.float32)
        parts = []
        for g in range(3):
            sl = slice(g * H + c * HC, g * H + (c + 1) * HC)
            parts.append(bi[sl] + bh[sl] if g < 2 else bi[sl])
        return np.concatenate(parts)[None, :].astype(bfnp)

    def bias_n(bh, c):
        bh = np.asarray(bh, np.float32)
        sl = slice(2 * H + c * HC, 2 * H + (c + 1) * HC)
        return bh[sl][None, :].astype(bfnp)

    def pack_kT(w_rows, kchunks, ncols):
        wT = np.ascontiguousarray(np.asarray(w_rows, np.float32).T)
        return np.ascontiguousarray(
            wT.reshape(kchunks, 128, ncols).transpose(1, 0, 2)
            .reshape(128, kchunks * ncols)).astype(bfnp)

    ones = np.ones((1, 512), np.float32).astype(bfnp)
    ident = np.eye(128, dtype=np.float32).astype(bfnp)
    dec_w = np.asarray(dec_w, np.float32)
    dec_b = np.asarray(dec_b, np.float32)
    in_maps = []
    for c in range(NCR):
        drows = slice(c * DC, (c + 1) * DC)
        in_maps.append({
            "xd": xd,
            "wih0T": pack_T(w_ih_l0, c, KX),
            "whh0T": pack_T(w_hh_l0, c, KH),
            "wih1T": pack_T(w_ih_l1, c, KH),
            "whh1T": pack_T(w_hh_l1, c, KH),
            "decw": pack_kT(dec_w[drows], KH, DC),
            "bih0": bias_mix(b_ih_l0, b_hh_l0, c),
            "b1x": bias_mix(b_ih_l1, b_hh_l1, c),
            "b0hn": bias_n(b_hh_l0, c),
            "b1hn": bias_n(b_hh_l1, c),
            "decb": dec_b[drows][None, :].astype(bfnp),
            "ones": ones,
            "ident": ident,
        })
    return in_maps


def kernel(**kw):
    key = "nc_v3"
    if key not in _CACHE:
        _CACHE[key] = _build()
    nc = _CACHE[key]
    _CACHE["nc"] = nc
    in_maps = _prep_in_maps(**kw)
    _CACHE["last_in_maps"] = in_maps
    res = bass_utils.run_bass_kernel_spmd(
        nc, in_maps, core_ids=list(range(NCR)), trace=False)
    out = np.concatenate([res.results[c]["out"] for c in range(NCR)], axis=1)
    return out
